# revision 100
# baseline (speedup 1.0000x reference)
"""Trainium2 Bass kernel for AdaptiveMHFConv (FNO-style spectral conv).

out = irfft2( pad_32x32( einsum('bhixy,hioxy', rfft2(x)[..., :32, :32], w) ) ) + bias

Sharding: 8 cores = 4 heads x 2 batch-halves; each core: 16 batches x 1 head
(16 in / 16 out channels), no collectives.

Per-core pipeline (modes 32x32, all DFTs are truncated-DFT matmuls):
  S1  per image: lhsT=x[h,w], rhs=EH[h,(kx-r|kx-i)] -> P1[w, (j8,ri2,kx32)]
  S2  lhsT=cos/sin/-sin[w,ky] (32 cols), rhs=P1s r/i col-sel
      -> XO[32ky, (ri2,j8,kx32)] (4 accumulating matmuls)
  CB  copy-scatter -> cb[b][32ky, (q8,kxl4,i16,ri2)]
  T1  8 PE-transposes -> mtall[h][128=(kxl,i,ri), (b8,q8,ky32)]
  S3  modal, per (q,ky,half): lhsT=WSLAB[u][128,128] block-diag complex
      (expanded on-chip from a 2MB compact DMA), rhs=mtall cols (8 b)
      -> mo[128=(o16,kxl4,ri2), (ky,b)] -> mos[h][128, (q,b,ky)]
  SLAB per (o,half): DMA repartition -> slab[64=(kxl,ri,q), (b,ky)]
  S4  lhsT=IEQR/IEQI[64,128h'], rhs=slab -> zrzi[h', (ri,b,ky)] -> zsg
  T3  PE-transposes zsg chunks -> zt[128=(b2,ri,ky), (ol,h')]
  S5  lhsT=CW[(ri,ky),w'], rhs=zt_sb -> y[w', (ol,h')] -> out DMA per b

Cost-model-driven choices: merged big DMAs (>=512B runs), g1-compact modal
weights + on-chip block-diag expansion, batch-half modal pipelining, copies
load-balanced across DVE/Act/Pool, PE warm-up to hold the 2.4GHz p-state.
"""
import os
import sys

import numpy as np

sys.path.insert(0, "/opt/trn_rl_repo")

import concourse.bass as bass  # noqa: E402
import concourse.mybir as mybir  # noqa: E402
from concourse.bass_utils import bass_rust, run_bass_kernel_spmd  # noqa: E402
from concourse.masks import make_identity  # noqa: E402
from concourse.tile import TileContext  # noqa: E402

F32 = mybir.dt.float32
F16 = mybir.dt.float16

WARMUP = 26
# SP-queue DMA order: b=blob, xN=x chunk, wN=wc1 chunk, sN=slab half-batch
DMA_ORDER = ["b", "w0", "x0", "x1", "x2", "w1", "x3", "x4", "x5",
             "x6", "x7"]


def _build_shared_consts():
    h = np.arange(128)
    k32 = np.arange(32)
    ang = 2 * np.pi * np.outer(h, k32) / 128.0
    EH = np.concatenate([np.cos(ang), -np.sin(ang)], axis=1)  # [128, 64]
    FW3 = np.concatenate([np.cos(ang), np.sin(ang), -np.sin(ang)], axis=1)
    CW = np.zeros((64, 128), np.float32)
    for ky in range(32):
        c = 1.0 if ky == 0 else 2.0
        a = 2 * np.pi * h * ky / 128.0
        CW[ky] = c * np.cos(a) / 128.0
        CW[32 + ky] = -c * np.sin(a) / 128.0
    CW[32] = 0.0  # irfft ignores Im of bin 0
    CW2 = np.concatenate([CW, CW], axis=0)  # [128, 128]
    IEQR = np.zeros((64, 128), np.float32)
    IEQI = np.zeros((64, 128), np.float32)
    for kxl in range(4):
        for q in range(8):
            kx = q * 4 + kxl
            a = 2 * np.pi * h * kx / 128.0
            IEQR[kxl * 16 + 0 * 8 + q] = np.cos(a) / 128.0
            IEQR[kxl * 16 + 1 * 8 + q] = -np.sin(a) / 128.0
            IEQI[kxl * 16 + 0 * 8 + q] = np.sin(a) / 128.0
            IEQI[kxl * 16 + 1 * 8 + q] = np.cos(a) / 128.0
    blob = np.zeros((128, 544), np.float32)
    blob[:, 0:64] = EH
    blob[:, 64:160] = FW3
    blob[:, 160:288] = CW2
    blob[0:64, 288:416] = IEQR
    blob[0:64, 416:544] = IEQI
    blob[64:128, 288:416] = IEQR
    blob[64:128, 416:544] = IEQI
    return blob.astype(np.float16)


def _build_wc1(w_real_h, w_imag_h):
    """[16i,16o,32kx,32ky] -> [128, 8192] g1-compact modal weights.

    rows (kxl4, i16, ri2); cols (u256=(q,ky), o16, ri'2)."""
    Wr = w_real_h.transpose(2, 3, 0, 1)  # [kx, ky, i, o]
    Wi = w_imag_h.transpose(2, 3, 0, 1)
    Wr = Wr.reshape(8, 4, 32, 16, 16).transpose(0, 2, 1, 3, 4)  # [q,ky,kxl,i,o]
    Wi = Wi.reshape(8, 4, 32, 16, 16).transpose(0, 2, 1, 3, 4)
    T = np.zeros((8, 32, 4, 16, 2, 16, 2), np.float32)  # [q,ky,kxl,i,ri,o,rp]
    T[..., 0, :, 0] = Wr
    T[..., 0, :, 1] = Wi
    T[..., 1, :, 0] = -Wi
    T[..., 1, :, 1] = Wr
    out = T.transpose(2, 3, 4, 0, 1, 5, 6).reshape(128, 8192)
    return np.ascontiguousarray(out).astype(np.float16)


class _Balancer:
    """Distribute copies across DVE/Act/Pool by modeled cost."""

    def __init__(self, nc):
        self.nc = nc
        self.load = {"v": 0.0, "s": 0.0, "p": 0.0}

    def _eng(self, e):
        return {"v": self.nc.vector, "s": self.nc.scalar,
                "p": self.nc.gpsimd}[e]

    def copy(self, out, in_, fsize, kind="f32psum", force=None):
        # GPSIMD (p) cannot access PSUM -> only SBUF->SBUF copies go there.
        if kind == "f32psum":
            costs = {"v": fsize * 0.9 + 125, "s": fsize * 0.95 + 150}
        elif kind == "f16psum":
            costs = {"v": fsize * 0.52 + 125, "s": fsize * 0.833 + 150}
        else:  # f16 sbuf->sbuf
            costs = {"v": fsize * 0.26 + 60, "s": fsize * 0.833 + 185,
                     "p": fsize * 1.389 + 60}
        e = force or min(costs, key=lambda k: self.load[k] + costs[k])
        self.load[e] += costs[e]
        eng = self._eng(e)
        if e == "s":
            eng.copy(out=out, in_=in_)
        else:
            eng.tensor_copy(out=out, in_=in_)
        return e

    def memset(self, ap, fsize, force=None):
        costs = {"v": fsize * 1.042 + 60, "p": fsize * 0.833 + 60}
        e = force or min(costs, key=lambda k: self.load[k] + costs[k])
        self.load[e] += costs[e]
        self._eng(e).memset(ap, 0.0)


PHASES = []


def _mark(nc, label):
    n = int(nc.get_next_instruction_name().split("-")[1])
    PHASES.append((label, n))


def _build_graph():
    nc = bass.Bass()
    x_ext = nc.declare_dram_parameter("x", [8, 128, 4096], F16, isOutput=False)
    blob_ext = nc.declare_dram_parameter("blob", [128, 544], F16,
                                         isOutput=False)
    wc1_ext = nc.declare_dram_parameter("wc1", [128, 8192], F16,
                                        isOutput=False)
    y_ext = nc.declare_dram_parameter("out", [16, 128, 2048], F16,
                                      isOutput=True)

    with TileContext(nc) as tc:
        bal = _Balancer(nc)
        with (
            tc.tile_pool(name="top", bufs=1) as tpool,
            tc.tile_pool(name="slabs", bufs=8) as slpool,
            tc.tile_pool(name="xts", bufs=5) as xpool,
        ):
            blob = tpool.tile([128, 544], F16, tag="blob")
            ident = tpool.tile([128, 128], F16, tag="ident")
            wc1 = tpool.tile([128, 8192], F16, tag="wc1")
            wslab = tpool.tile([128, 32768], F16, tag="wslab")
            mtall = tpool.tile([128, 4096], F16, tag="mtall")
            mos = tpool.tile([128, 4096], F16, tag="mos")

            # ---- SP DMA stream in explicit order ----
            xts = [None] * 8
            slabs = {}

            def emit_dma(item):
                if item == "b":
                    nc.sync.dma_start(out=blob, in_=blob_ext[:])
                elif item[0] == "x":
                    c = int(item[1:])
                    xt = xpool.tile([128, 4096], F16, tag="xt",
                                    name=f"xt{c}")
                    nc.sync.dma_start(out=xt, in_=x_ext[c])
                    xts[c] = xt
                elif item[0] == "w":
                    c = int(item[1:])
                    nc.sync.dma_start(out=wc1[:, c * 4096:(c + 1) * 4096],
                                      in_=wc1_ext[:, c * 4096:(c + 1) * 4096])
                elif item[0] == "s":
                    half = int(item[1:])
                    eng = nc.sync if half == 0 else nc.gpsimd
                    for op in range(half * 4, half * 4 + 4):
                        sl = slpool.tile([128, 512], F16, tag="slab",
                                         name=f"slab{op}")
                        eng.dma_start(
                            out=sl.rearrange("p (b y) -> p b y", b=16),
                            in_=mos[op * 16:op * 16 + 16, :].rearrange(
                                "p (q b y) -> p q b y", q=8, b=16))
                        slabs[op] = sl

            for item in DMA_ORDER:
                emit_dma(item)
            make_identity(nc, ident)

            def wslab_memset(c):
                # f32 bitcast halves the modeled per-element memset cost
                bal.memset(wslab[:, c * 2048:(c + 1) * 2048].bitcast(F32),
                           1024, force="p")

            def expand_chunk(c, force=None):
                """wc1 u-chunk c (16 u-tiles) -> block-diag wslab cols."""
                for kl in range(4):
                    out_v = wslab[kl * 32:(kl + 1) * 32, :].rearrange(
                        "p (u o k r) -> p u o k r", u=256, o=16,
                        k=4)[:, c * 16:(c + 1) * 16, :, kl]
                    in_v = wc1[kl * 32:(kl + 1) * 32,
                               c * 512:(c + 1) * 512].rearrange(
                        "p (u o r) -> p u o r", u=16, o=16)
                    bal.copy(out_v, in_v, 512, kind="f16sbuf", force=force)

            mtv_all = mtall.rearrange("p (b u) -> p u b", u=256)

            # ================= Phase F =================
            for t in range(16):
                wslab_memset(t)

            with (
                tc.tile_pool(name="fsb", bufs=5) as fsb,
                tc.tile_pool(name="psF1", bufs=3, space="PSUM") as psF1,
                tc.tile_pool(name="psF2", bufs=2, space="PSUM") as psF2,
                tc.tile_pool(name="psFt", bufs=1, space="PSUM") as psFt,
            ):
                for w in range(WARMUP):
                    wm = psF1.tile([128, 512], F32, tag="p1", name=f"wm{w}")
                    nc.tensor.matmul(wm[:, 0:256], lhsT=blob[:, 0:128],
                                     rhs=blob[:, 0:256], start=True, stop=True)

                p1s_q = {}
                cb_q = {}

                def S1(b):
                    c, off = b // 2, (b % 2) * 2048
                    p1s = fsb.tile([128, 1024], F16, tag="p1s",
                                   name=f"p1s_{b}")
                    for g in range(2):
                        p1 = psF1.tile([128, 512], F32, tag="p1",
                                       name=f"p1_{b}_{g}")
                        for j in range(8):
                            nc.tensor.matmul(
                                p1[:, j * 64:(j + 1) * 64],
                                lhsT=xts[c][:, off + g * 1024 + j * 128:
                                            off + g * 1024 + (j + 1) * 128],
                                rhs=blob[:, 0:64], start=True, stop=True)
                        bal.copy(p1s[:, g * 512:(g + 1) * 512], p1, 512)
                    p1s_q[b] = p1s

                def S2(b):
                    cb = fsb.tile([32, 1024], F16, tag="cb", name=f"cb{b}")
                    xo = psF2.tile([32, 1024], F32, tag="xo", name=f"xo_{b}",
                                   bufs=2)
                    p1s = p1s_q.pop(b)
                    selg = p1s.rearrange("p (g j r k) -> p g r j k",
                                         g=2, j=8, r=2)
                    for g in range(2):
                        rsel, isel = selg[:, g, 0], selg[:, g, 1]
                        c0 = g * 512
                        nc.tensor.matmul(xo[:, c0:c0 + 256],
                                         lhsT=blob[:, 64:96],
                                         rhs=rsel, start=True, stop=False)
                        nc.tensor.matmul(xo[:, c0:c0 + 256],
                                         lhsT=blob[:, 96:128],
                                         rhs=isel, start=False, stop=True)
                        nc.tensor.matmul(xo[:, c0 + 256:c0 + 512],
                                         lhsT=blob[:, 64:96],
                                         rhs=isel, start=True, stop=False)
                        nc.tensor.matmul(xo[:, c0 + 256:c0 + 512],
                                         lhsT=blob[:, 128:160],
                                         rhs=rsel, start=False, stop=True)
                    out_v = cb.rearrange("p (m G j r) -> p G m j r",
                                         m=32, G=2, j=8)
                    in_v = xo.rearrange("p (G r j m) -> p G m j r",
                                        G=2, r=2, j=8)
                    bal.copy(out_v[:, 0], in_v[:, 0], 512, force="s")
                    bal.copy(out_v[:, 1], in_v[:, 1], 512, force="v")
                    cb_q[b] = cb

                def T1(b):
                    cb = cb_q.pop(b)
                    pt = psFt.tile([128, 256], F16, tag="pt", name=f"pt{b}")
                    for q in range(8):
                        nc.tensor.transpose(pt[:, q * 32:(q + 1) * 32],
                                            cb[:, q * 128:(q + 1) * 128],
                                            ident[0:32, 0:32])
                    bal.copy(mtall[:, b * 256:(b + 1) * 256],
                             pt, 256, kind="f16psum")

                _mark(nc, "F")
                EXP_ENG = {0: "v", 1: "p", 2: "s", 3: "p"}
                for t in range(18):
                    _mark(nc, f"F{t}")
                    if t < 4:
                        cs = (2 * t, 2 * t + 1)
                    elif 6 <= t < 10:
                        cs = (2 * (t - 6) + 8, 2 * (t - 6) + 9)
                    else:
                        cs = ()
                    for c in cs:
                        expand_chunk(c, force=EXP_ENG[c % 4])
                    if t < 16:
                        S1(t)
                    if 1 <= t < 17:
                        S2(t - 1)
                    if t >= 2:
                        T1(t - 2)

            # ================= Phase M =================
            _mark(nc, "M")
            with tc.tile_pool(name="psM", bufs=4, space="PSUM") as psM:
                for q in range(8):
                    for kh in range(2):
                        mo = psM.tile([128, 256], F32, tag="mo",
                                      name=f"mo{q}_{kh}")
                        for s in range(16):
                            ky = kh * 16 + s
                            u = q * 32 + ky
                            nc.tensor.matmul(
                                mo[:, s * 16:(s + 1) * 16],
                                lhsT=wslab[:, u * 128:(u + 1) * 128],
                                rhs=mtv_all[:, u, :], start=True, stop=True)
                        out_v = mos[:, q * 512:(q + 1) * 512].rearrange(
                            "p (b y) -> p y b", b=16)[:, kh * 16:
                                                     (kh + 1) * 16]
                        in_v = mo.rearrange("p (y b) -> p y b", y=16)
                        bal.copy(out_v, in_v, 256,
                                 force=("s" if (2 * q + kh) % 2 else "v"))
                emit_dma("s0")
                emit_dma("s1")
            _mark(nc, "slab")

            # ================= Phase I =================
            with tc.tile_pool(name="isb", bufs=1) as isb:
                _mark(nc, "IA")
                zsgs = []
                with tc.tile_pool(name="psI4", bufs=4, space="PSUM") as psI4:
                    for og in range(4):
                        zsg = isb.tile([128, 4096], F16, tag="zsg", bufs=4,
                                       name=f"zsg{og}")
                        zv = zsg.rearrange("p (l b r y) -> p l b r y",
                                           l=4, b=16, r=2)
                        for ol in range(4):
                            o = og * 4 + ol
                            rb = (o % 2) * 64
                            sl = slabs[o // 2][rb:rb + 64, :]
                            zrzi = psI4.tile([128, 1024], F32, tag="zrzi",
                                             name=f"zrzi{o}")
                            nc.tensor.matmul(zrzi[:, 0:512],
                                             lhsT=blob[rb:rb + 64, 288:416],
                                             rhs=sl, start=True, stop=True)
                            nc.tensor.matmul(zrzi[:, 512:1024],
                                             lhsT=blob[rb:rb + 64, 416:544],
                                             rhs=sl, start=True, stop=True)
                            bal.copy(zv[:, ol],
                                     zrzi.rearrange("p (r b y) -> p b r y",
                                                    r=2, b=16), 1024)
                        zsgs.append(zsg)

                _mark(nc, "IB")
                psT3_cm = tc.tile_pool(name="psT3", bufs=2, space="PSUM")
                psT3 = psT3_cm.__enter__()
                psY_cm = tc.tile_pool(name="psY", bufs=3, space="PSUM")
                psY = psY_cm.__enter__()
                zt_q = {}
                for p in range(9):
                    for gg in range(2):
                        if p < 8:
                            zt = psT3.tile([128, 1024], F16, tag="zt",
                                           name=f"zt_{p}_{gg}")
                            for k in range(8):
                                og, ol = gg * 2 + k // 4, k % 4
                                nc.tensor.transpose(
                                    zt[:, k * 128:(k + 1) * 128],
                                    zsgs[og][:, ol * 1024 + p * 128:
                                             ol * 1024 + (p + 1) * 128],
                                    ident)
                            zt_sb = isb.tile([128, 1024], F16, tag="ztsb",
                                             bufs=6, name=f"ztsb_{p}_{gg}")
                            bal.copy(zt_sb, zt, 1024, kind="f16psum", force="v")
                            zt_q[(p, gg)] = zt_sb
                        if p >= 1:
                            pp = p - 1
                            ztsb = zt_q.pop((pp, gg))
                            for db in range(2):
                                b = pp * 2 + db
                                rb = db * 64
                                out_t = isb.tile([128, 1024], F16,
                                                 tag="outsb", bufs=8,
                                                 name=f"out{b}_{gg}")
                                y = psY.tile([128, 1024], F32, tag="y",
                                             name=f"y_{b}_{gg}")
                                for half in range(2):
                                    nc.tensor.matmul(
                                        y[:, half * 512:(half + 1) * 512],
                                        lhsT=blob[rb:rb + 64, 160:288],
                                        rhs=ztsb[rb:rb + 64,
                                                 half * 512:
                                                 (half + 1) * 512],
                                        start=True, stop=True)
                                bal.copy(out_t, y, 1024)
                                nc.sync.dma_start(
                                    out=y_ext[b, :, gg * 1024:(gg + 1) * 1024],
                                    in_=out_t)
                psY_cm.__exit__(None, None, None)
                psT3_cm.__exit__(None, None, None)

    bass_rust.move_matmul_waits_to_ldweights(nc.m)
    bass_rust.generate_event_semaphores(nc)
    return nc


_CACHE = {}


def kernel(x, w_real, w_imag, bias):
    x = np.asarray(x, np.float32)
    w_real = np.asarray(w_real, np.float32)
    w_imag = np.asarray(w_imag, np.float32)
    bias = np.asarray(bias, np.float32)

    if "nc" not in _CACHE:
        _CACHE["nc"] = _build_graph()
        _CACHE["blob"] = _build_shared_consts()
    nc = _CACHE["nc"]
    blob = _CACHE["blob"]

    in_maps = []
    for c in range(8):
        head, half = c // 2, c % 2
        xs = x[half * 16:(half + 1) * 16, head * 16:(head + 1) * 16]
        xt = np.ascontiguousarray(
            xs.reshape(16, 2, 8, 128, 128).transpose(0, 1, 3, 2, 4)
        ).reshape(32, 128, 1024).reshape(8, 4, 128, 1024).transpose(
            0, 2, 1, 3).reshape(8, 128, 4096)
        xt = np.ascontiguousarray(xt).astype(np.float16)
        key = ("wc1", head)
        if key not in _CACHE:
            _CACHE[key] = _build_wc1(w_real[head], w_imag[head])
        in_maps.append({"x": xt, "blob": blob, "wc1": _CACHE[key]})

    trace = os.environ.get("KERNEL_TRACE", "0") == "1"
    res = run_bass_kernel_spmd(nc, in_maps, core_ids=list(range(8)),
                               trace=trace)
    _CACHE["exec_time_ns"] = res.exec_time_ns

    out = np.empty((32, 64, 128, 128), np.float32)
    for c in range(8):
        head, half = c // 2, c % 2
        # res [16b, 128 w', (o16, h'128)] -> [b, o, h, w]
        ys = res.results[c]["out"].astype(np.float32).reshape(
            16, 128, 16, 128).transpose(0, 2, 3, 1)
        out[half * 16:(half + 1) * 16, head * 16:(head + 1) * 16] = ys
    return out + bias[None]
